# revision 11
# baseline (speedup 1.0000x reference)
"""Trainium2 Bass kernel for nn_DenseCondenser (TT contraction, 65536x4096 -> 65536x8).

The three (8,8,8) TT cores compose into a single effective matrix E (4096, 8)
(the whole map is linear in x), folded on host in float64. The device kernel
is a memory-bound skinny matmul out = x @ E + bias, data-parallel over the
batch across 8 NeuronCores.

v2: x is quantized to fp8 e3m4 on host (1 B/elem -> HBM traffic 33.5 MB/core,
~4x less than fp32), E stays bf16 (mixed-dtype matmul; e3m4's range can't hold
E's ~1e-2 magnitudes). Measured end-to-end L2 rel err ~1.34e-2, dominated by
the e3m4 rounding of x (gate is 2e-2; the error is deterministic).

The PE contraction-feed bound at 1 col/cycle (109 us/core) would exceed the
fp8 DMA time (~91 us), so the 32 k-tiles are spread over 4 PE column groups
(tile_position=(0,32g)) that run concurrently - stationary E is only 8 wide.
Each group accumulates its 8 k-tiles into its own PSUM partition range;
VectorE folds the 4 partials + bias into the output tile. PE span per chunk
is ~1.7 us (warm) vs 5.7 us of DMA, so the kernel stays DMA-bound even with
a cold (HAM-throttled) PE.

Device layout: xb8 (16 chunks, 128 partitions, 32 ktiles, 512 batch) fp8 so
the contraction dim lands on SBUF partitions and every (chunk, partition) DMA
payload is one contiguous 16 KiB run. Stores ride the Scalar HWDGE ring,
grouped (8, 2048).
"""

import numpy as np

import concourse.bass as bass
import concourse.mybir as mybir
import concourse.tile as tile
from concourse import bacc
from concourse.bass import ts
from concourse.bass_utils import run_bass_kernel_spmd

# Problem shapes (hardcoded per harness contract)
BATCH = 65536
K = 4096  # input features = 8**4
C = 8  # output features
N_CORES = 8
B_CORE = BATCH // N_CORES  # 8192
CHUNK = 512  # batch columns per matmul (one PSUM bank at fp32)
NK = K // 128  # 32 k-tiles
NCHUNK = B_CORE // CHUNK  # 16
NGRP = 4  # PE column groups running concurrently
KT_PER_GRP = NK // NGRP  # 8

_program_cache = {}


def _build_program() -> bass.Bass:
    f32 = mybir.dt.float32
    bf16 = mybir.dt.bfloat16
    fp8 = mybir.dt.float8e3
    add = mybir.AluOpType.add
    mult = mybir.AluOpType.mult
    nc = bacc.Bacc(None, name="dense_condenser")

    # xb8[j, p, kt, b] = x[j*CHUNK + b, kt*128 + p]: per (chunk, partition)
    # the (kt, b) payload is one contiguous 16 KiB run -> max DMA efficiency.
    xb8 = nc.dram_tensor("xb8", (NCHUNK, 128, NK, CHUNK), fp8, kind="ExternalInput")
    eb = nc.dram_tensor("eb", (128, NK, C), bf16, kind="ExternalInput")
    bias = nc.dram_tensor("bias", (C, 1), f32, kind="ExternalInput")
    outT = nc.dram_tensor("outT", (C, B_CORE), f32, kind="ExternalOutput")

    with tile.TileContext(nc) as tc:
        with (
            tc.tile_pool(name="consts", bufs=1) as consts,
            tc.tile_pool(name="xp", bufs=3) as xp,
            tc.tile_pool(name="tp", bufs=3) as tp,
            tc.tile_pool(name="op", bufs=2) as op,
            tc.tile_pool(name="pp", bufs=4, space=bass.MemorySpace.PSUM) as pp,
        ):
            e_tile = consts.tile([128, NK, C], bf16)
            bias_tile = consts.tile([C, 1], f32)
            # consts ride the Scalar DGE ring (idle until the first store)
            # so the Sync ring's first x trigger isn't queued behind them
            nc.scalar.dma_start(out=e_tile[:], in_=eb[:])
            nc.scalar.dma_start(out=bias_tile[:], in_=bias[:])

            GROUP = 2  # output chunks per store
            out_tile = None
            for j in range(NCHUNK):
                x_tile = xp.tile([128, NK, CHUNK], fp8)
                # quarter-loads: matmuls trail each quarter, so only 8
                # matmuls remain once the last bytes land
                NQ = NK // 4
                for q in range(4):
                    nc.sync.dma_start(
                        out=x_tile[:, q * NQ : (q + 1) * NQ],
                        in_=xb8[j, :, q * NQ : (q + 1) * NQ],
                    )

                # 4 col-groups accumulate disjoint k-tile subsets into
                # disjoint PSUM partition ranges, concurrently on the PE.
                psum_tile = pp.tile([128, CHUNK], f32)
                for t in range(KT_PER_GRP):
                    for g in range(NGRP):
                        kt = t * NGRP + g
                        nc.tensor.matmul(
                            psum_tile[32 * g : 32 * g + C],
                            e_tile[:, kt, :],
                            x_tile[:, kt, :],
                            start=(t == 0),
                            stop=(t == KT_PER_GRP - 1),
                            tile_position=(0, 32 * g),
                        )

                if j % GROUP == 0:
                    out_tile = op.tile([C, GROUP * CHUNK], f32, tag="out")
                # fold the 4 partials + bias. TensorTensor can read only one
                # PSUM input per op, so chain s1 = psum_g0 + bias;
                # s_k = psum_gk + s_{k-1}. The first step runs as a ScalarE
                # activation (identity, per-partition bias) so VectorE's
                # serial chain is 3 ops, not 4.
                t1 = tp.tile([C, CHUNK], f32)
                t2 = tp.tile([C, CHUNK], f32)
                nc.scalar.activation(
                    t1[:],
                    psum_tile[0:C],
                    mybir.ActivationFunctionType.Identity,
                    bias=bias_tile[:],
                )
                nc.vector.scalar_tensor_tensor(
                    t2[:], psum_tile[32 : 32 + C], 1.0, t1[:], mult, add
                )
                nc.vector.scalar_tensor_tensor(
                    t1[:], psum_tile[64 : 64 + C], 1.0, t2[:], mult, add
                )
                nc.vector.scalar_tensor_tensor(
                    out_tile[:, ts(j % GROUP, CHUNK)],
                    psum_tile[96 : 96 + C],
                    1.0,
                    t1[:],
                    mult,
                    add,
                )
                if j % GROUP == GROUP - 1:
                    # stores ride the Scalar HWDGE ring, never stalling the
                    # Sync ring that feeds the streaming loads
                    nc.scalar.dma_start(
                        out=outT[:, ts(j // GROUP, GROUP * CHUNK)], in_=out_tile[:]
                    )

    nc.compile()
    return nc


def _fold_E(node_0, node_1, node_2) -> np.ndarray:
    # E[(i,j,k,l), c3] = sum_{c1,c2} node_0[l,k,c1] node_1[c1,j,c2] node_2[c2,i,c3]
    E = np.einsum(
        "lkc,cjd,die->ijkle",
        node_0.astype(np.float64),
        node_1.astype(np.float64),
        node_2.astype(np.float64),
    )
    return E.reshape(K, C).astype(np.float32)


def kernel(x, node_0, node_1, node_2, bias, _trace=False, _trace_cores=None):
    import ml_dtypes

    x = np.asarray(x, dtype=np.float32)
    E = _fold_E(np.asarray(node_0), np.asarray(node_1), np.asarray(node_2))
    bias_np = np.asarray(bias, dtype=np.float32).reshape(C, 1)

    # blocked E: eb[p, kt, c] = E[kt*128 + p, c]
    eb = np.ascontiguousarray(E.reshape(NK, 128, C).transpose(1, 0, 2)).astype(
        ml_dtypes.bfloat16
    )

    if "v2" not in _program_cache:
        _program_cache["v2"] = _build_program()
    nc = _program_cache["v2"]

    # quantize once in the natural layout, then do the (cheaper) 1-byte
    # blocked transpose per core
    x8 = x.astype(ml_dtypes.float8_e3m4)

    in_maps = []
    for m in range(N_CORES):
        x_m = x8[m * B_CORE : (m + 1) * B_CORE, :]
        # xb8[j, p, kt, b] = x_m[j*CHUNK + b, kt*128 + p]
        xb_m = np.ascontiguousarray(
            x_m.reshape(NCHUNK, CHUNK, NK, 128).transpose(0, 3, 2, 1)
        )
        in_maps.append({"xb8": xb_m, "eb": eb, "bias": bias_np})

    res = run_bass_kernel_spmd(
        nc,
        in_maps,
        core_ids=list(range(N_CORES)),
        trace=_trace,
        trace_cores=_trace_cores,
    )
    results = res.results

    out = np.empty((BATCH, C), dtype=np.float32)
    for m in range(N_CORES):
        out[m * B_CORE : (m + 1) * B_CORE, :] = results[m]["outT"].T

    if _trace:
        return out, res
    return out


# revision 12
# speedup vs baseline: 1.0330x; 1.0330x over previous
"""Trainium2 Bass kernel for nn_DenseCondenser (TT contraction, 65536x4096 -> 65536x8).

The three (8,8,8) TT cores compose into a single effective matrix E (4096, 8)
(the whole map is linear in x), folded on host in float64. The device kernel
is a memory-bound skinny matmul out = x @ E + bias, data-parallel over the
batch across 8 NeuronCores.

v2: x is quantized to fp8 e3m4 on host (1 B/elem -> HBM traffic 33.5 MB/core,
~4x less than fp32), E stays bf16 (mixed-dtype matmul; e3m4's range can't hold
E's ~1e-2 magnitudes). Measured end-to-end L2 rel err ~1.34e-2, dominated by
the e3m4 rounding of x (gate is 2e-2; the error is deterministic).

The PE contraction-feed bound at 1 col/cycle (109 us/core) would exceed the
fp8 DMA time (~91 us), so the 32 k-tiles are spread over 4 PE column groups
(tile_position=(0,32g)) that run concurrently - stationary E is only 8 wide.
Each group accumulates its 8 k-tiles into its own PSUM partition range;
VectorE folds the 4 partials + bias into the output tile. PE span per chunk
is ~1.7 us (warm) vs 5.7 us of DMA, so the kernel stays DMA-bound even with
a cold (HAM-throttled) PE.

Device layout: xb8 (16 chunks, 128 partitions, 32 ktiles, 512 batch) fp8 so
the contraction dim lands on SBUF partitions and every (chunk, partition) DMA
payload is one contiguous 16 KiB run. Stores ride the Scalar HWDGE ring,
grouped (8, 2048).
"""

import numpy as np

import concourse.bass as bass
import concourse.mybir as mybir
import concourse.tile as tile
from concourse import bacc
from concourse.bass import ts
from concourse.bass_utils import run_bass_kernel_spmd

# Problem shapes (hardcoded per harness contract)
BATCH = 65536
K = 4096  # input features = 8**4
C = 8  # output features
N_CORES = 8
B_CORE = BATCH // N_CORES  # 8192
CHUNK = 512  # batch columns per matmul (one PSUM bank at fp32)
NK = K // 128  # 32 k-tiles
NCHUNK = B_CORE // CHUNK  # 16
NGRP = 4  # PE column groups running concurrently
KT_PER_GRP = NK // NGRP  # 8

_program_cache = {}


def _build_program() -> bass.Bass:
    f32 = mybir.dt.float32
    bf16 = mybir.dt.bfloat16
    fp8 = mybir.dt.float8e3
    add = mybir.AluOpType.add
    mult = mybir.AluOpType.mult
    nc = bacc.Bacc(None, name="dense_condenser")

    # xb8[j, p, kt, b] = x[j*CHUNK + b, kt*128 + p]: per (chunk, partition)
    # the (kt, b) payload is one contiguous 16 KiB run -> max DMA efficiency.
    xb8 = nc.dram_tensor("xb8", (NCHUNK, 128, NK, CHUNK), fp8, kind="ExternalInput")
    eb = nc.dram_tensor("eb", (128, NK, C), bf16, kind="ExternalInput")
    bias = nc.dram_tensor("bias", (C, 1), f32, kind="ExternalInput")
    outT = nc.dram_tensor("outT", (C, B_CORE), f32, kind="ExternalOutput")

    with tile.TileContext(nc) as tc:
        with (
            tc.tile_pool(name="consts", bufs=1) as consts,
            tc.tile_pool(name="xp", bufs=3) as xp,
            tc.tile_pool(name="tp", bufs=3) as tp,
            tc.tile_pool(name="op", bufs=2) as op,
            tc.tile_pool(name="pp", bufs=4, space=bass.MemorySpace.PSUM) as pp,
        ):
            e_tile = consts.tile([128, NK, C], bf16)
            bias_tile = consts.tile([C, 1], f32)
            # consts ride the Scalar DGE ring (idle until the first store)
            # so the Sync ring's first x trigger isn't queued behind them
            nc.scalar.dma_start(out=e_tile[:], in_=eb[:])
            nc.scalar.dma_start(out=bias_tile[:], in_=bias[:])

            GROUP = 2  # output chunks per store
            out_tile = None
            for j in range(NCHUNK):
                x_tile = xp.tile([128, NK, CHUNK], fp8)
                # two half-loads: matmuls on the first half overlap the
                # second half's DMA. (Quarter-loads measured slower: 4 KiB
                # per-partition descriptors drop HBM rate 362 -> 326 GB/s.)
                nc.sync.dma_start(out=x_tile[:, : NK // 2], in_=xb8[j, :, : NK // 2])
                nc.sync.dma_start(out=x_tile[:, NK // 2 :], in_=xb8[j, :, NK // 2 :])

                # 4 col-groups accumulate disjoint k-tile subsets into
                # disjoint PSUM partition ranges, concurrently on the PE.
                psum_tile = pp.tile([128, CHUNK], f32)
                for t in range(KT_PER_GRP):
                    for g in range(NGRP):
                        kt = t * NGRP + g
                        nc.tensor.matmul(
                            psum_tile[32 * g : 32 * g + C],
                            e_tile[:, kt, :],
                            x_tile[:, kt, :],
                            start=(t == 0),
                            stop=(t == KT_PER_GRP - 1),
                            tile_position=(0, 32 * g),
                        )

                if j % GROUP == 0:
                    out_tile = op.tile([C, GROUP * CHUNK], f32, tag="out")
                # fold the 4 partials + bias. TensorTensor can read only one
                # PSUM input per op, so chain s1 = psum_g0 + bias;
                # s_k = psum_gk + s_{k-1}. The first step runs as a ScalarE
                # activation (identity, per-partition bias) so VectorE's
                # serial chain is 3 ops, not 4.
                t1 = tp.tile([C, CHUNK], f32)
                t2 = tp.tile([C, CHUNK], f32)
                nc.scalar.activation(
                    t1[:],
                    psum_tile[0:C],
                    mybir.ActivationFunctionType.Identity,
                    bias=bias_tile[:],
                )
                nc.vector.scalar_tensor_tensor(
                    t2[:], psum_tile[32 : 32 + C], 1.0, t1[:], mult, add
                )
                nc.vector.scalar_tensor_tensor(
                    t1[:], psum_tile[64 : 64 + C], 1.0, t2[:], mult, add
                )
                nc.vector.scalar_tensor_tensor(
                    out_tile[:, ts(j % GROUP, CHUNK)],
                    psum_tile[96 : 96 + C],
                    1.0,
                    t1[:],
                    mult,
                    add,
                )
                if j % GROUP == GROUP - 1:
                    # stores ride the Scalar HWDGE ring, never stalling the
                    # Sync ring that feeds the streaming loads
                    nc.scalar.dma_start(
                        out=outT[:, ts(j // GROUP, GROUP * CHUNK)], in_=out_tile[:]
                    )

    nc.compile()
    return nc


def _fold_E(node_0, node_1, node_2) -> np.ndarray:
    # E[(i,j,k,l), c3] = sum_{c1,c2} node_0[l,k,c1] node_1[c1,j,c2] node_2[c2,i,c3]
    E = np.einsum(
        "lkc,cjd,die->ijkle",
        node_0.astype(np.float64),
        node_1.astype(np.float64),
        node_2.astype(np.float64),
    )
    return E.reshape(K, C).astype(np.float32)


def kernel(x, node_0, node_1, node_2, bias, _trace=False, _trace_cores=None):
    import ml_dtypes

    x = np.asarray(x, dtype=np.float32)
    E = _fold_E(np.asarray(node_0), np.asarray(node_1), np.asarray(node_2))
    bias_np = np.asarray(bias, dtype=np.float32).reshape(C, 1)

    # blocked E: eb[p, kt, c] = E[kt*128 + p, c]
    eb = np.ascontiguousarray(E.reshape(NK, 128, C).transpose(1, 0, 2)).astype(
        ml_dtypes.bfloat16
    )

    if "v2" not in _program_cache:
        _program_cache["v2"] = _build_program()
    nc = _program_cache["v2"]

    # quantize once in the natural layout, then do the (cheaper) 1-byte
    # blocked transpose per core
    x8 = x.astype(ml_dtypes.float8_e3m4)

    in_maps = []
    for m in range(N_CORES):
        x_m = x8[m * B_CORE : (m + 1) * B_CORE, :]
        # xb8[j, p, kt, b] = x_m[j*CHUNK + b, kt*128 + p]
        xb_m = np.ascontiguousarray(
            x_m.reshape(NCHUNK, CHUNK, NK, 128).transpose(0, 3, 2, 1)
        )
        in_maps.append({"xb8": xb_m, "eb": eb, "bias": bias_np})

    res = run_bass_kernel_spmd(
        nc,
        in_maps,
        core_ids=list(range(N_CORES)),
        trace=_trace,
        trace_cores=_trace_cores,
    )
    results = res.results

    out = np.empty((BATCH, C), dtype=np.float32)
    for m in range(N_CORES):
        out[m * B_CORE : (m + 1) * B_CORE, :] = results[m]["outT"].T

    if _trace:
        return out, res
    return out


# revision 15
# speedup vs baseline: 1.1590x; 1.1220x over previous
"""Trainium2 Bass kernel for nn_DenseCondenser (TT contraction, 65536x4096 -> 65536x8).

The three (8,8,8) TT cores compose into a single effective matrix E (4096, 8)
(the whole map is linear in x), folded on host in float64. The device kernel
is a memory-bound skinny matmul out = x @ E + bias, data-parallel over the
batch across 8 NeuronCores.

v2: x is quantized to fp8 e3m4 on host (1 B/elem -> HBM traffic 33.5 MB/core,
~4x less than fp32), E stays bf16 (mixed-dtype matmul; e3m4's range can't hold
E's ~1e-2 magnitudes). Measured end-to-end L2 rel err ~1.34e-2, dominated by
the e3m4 rounding of x (gate is 2e-2; the error is deterministic).

The PE contraction-feed bound at 1 col/cycle (109 us/core) would exceed the
fp8 DMA time (~91 us), so the 32 k-tiles are spread over 4 PE column groups
(tile_position=(0,32g)) that run concurrently - stationary E is only 8 wide.
Each group accumulates its 8 k-tiles into its own PSUM partition range;
VectorE folds the 4 partials + bias into the output tile. PE span per chunk
is ~1.7 us (warm) vs 5.7 us of DMA, so the kernel stays DMA-bound even with
a cold (HAM-throttled) PE.

Device layout: xb8 (16 chunks, 128 partitions, 32 ktiles, 512 batch) fp8 so
the contraction dim lands on SBUF partitions and every (chunk, partition) DMA
payload is one contiguous 16 KiB run. Stores ride the Scalar HWDGE ring,
grouped (8, 2048).
"""

import numpy as np

import concourse.bass as bass
import concourse.mybir as mybir
import concourse.tile as tile
from concourse import bacc
from concourse.bass import ts
from concourse.bass_utils import run_bass_kernel_spmd

# Problem shapes (hardcoded per harness contract)
BATCH = 65536
K = 4096  # input features = 8**4
C = 8  # output features
N_CORES = 8
B_CORE = BATCH // N_CORES  # 8192
CHUNK = 512  # batch columns per matmul (one PSUM bank at fp32)
NK = K // 128  # 32 k-tiles
NCHUNK = B_CORE // CHUNK  # 16
NGRP = 4  # PE column groups running concurrently
KT_PER_GRP = NK // NGRP  # 8

_program_cache = {}


def _build_program() -> bass.Bass:
    f32 = mybir.dt.float32
    bf16 = mybir.dt.bfloat16
    fp8 = mybir.dt.float8e3
    add = mybir.AluOpType.add
    mult = mybir.AluOpType.mult
    nc = bacc.Bacc(None, name="dense_condenser")

    # xb8[j, p, kt, b] = x[j*CHUNK + b, kt*128 + p]: per (chunk, partition)
    # the (kt, b) payload is one contiguous 16 KiB run -> max DMA efficiency.
    xb8 = nc.dram_tensor("xb8", (NCHUNK, 128, NK, CHUNK), fp8, kind="ExternalInput")
    eb = nc.dram_tensor("eb", (128, NK, C), bf16, kind="ExternalInput")
    bias = nc.dram_tensor("bias", (C, 1), f32, kind="ExternalInput")
    outT = nc.dram_tensor("outT", (C, B_CORE), f32, kind="ExternalOutput")

    with tile.TileContext(nc) as tc:
        with (
            tc.tile_pool(name="consts", bufs=1) as consts,
            tc.tile_pool(name="xp", bufs=4) as xp,
            tc.tile_pool(name="tp", bufs=3) as tp,
            tc.tile_pool(name="op", bufs=2) as op,
            tc.tile_pool(name="pp", bufs=4, space=bass.MemorySpace.PSUM) as pp,
        ):
            e_tile = consts.tile([128, NK, C], bf16)
            bias_tile = consts.tile([C, 1], f32)
            # consts ride the Scalar DGE ring (idle until the first store)
            # so the Sync ring's first x trigger isn't queued behind them
            nc.scalar.dma_start(out=e_tile[:], in_=eb[:])
            nc.scalar.dma_start(out=bias_tile[:], in_=bias[:])

            GROUP = 1  # output chunks per store
            out_tile = None
            for j in range(NCHUNK):
                x_tile = xp.tile([128, NK, CHUNK], fp8)
                # two half-loads: matmuls on the first half overlap the
                # second half's DMA. (Quarter-loads measured slower: 4 KiB
                # per-partition descriptors drop HBM rate 362 -> 326 GB/s.)
                # Last chunk splits 24/8 so only 8 matmuls trail the final
                # bytes, shortening the tail.
                SPLIT = NK // 2 if j < NCHUNK - 1 else 3 * NK // 4
                nc.sync.dma_start(out=x_tile[:, :SPLIT], in_=xb8[j, :, :SPLIT])
                nc.sync.dma_start(out=x_tile[:, SPLIT:], in_=xb8[j, :, SPLIT:])

                # 4 col-groups accumulate disjoint k-tile subsets into
                # disjoint PSUM partition ranges, concurrently on the PE.
                psum_tile = pp.tile([128, CHUNK], f32)
                for t in range(KT_PER_GRP):
                    for g in range(NGRP):
                        kt = t * NGRP + g
                        nc.tensor.matmul(
                            psum_tile[32 * g : 32 * g + C],
                            e_tile[:, kt, :],
                            x_tile[:, kt, :],
                            start=(t == 0),
                            stop=(t == KT_PER_GRP - 1),
                            tile_position=(0, 32 * g),
                        )

                if j % GROUP == 0:
                    out_tile = op.tile([C, GROUP * CHUNK], f32, tag="out")
                # fold the 4 partials + bias. TensorTensor can read only one
                # PSUM input per op, so chain s1 = psum_g0 + bias;
                # s_k = psum_gk + s_{k-1}. The first step runs as a ScalarE
                # activation (identity, per-partition bias) so VectorE's
                # serial chain is 3 ops, not 4.
                t1 = tp.tile([C, CHUNK], f32)
                t2 = tp.tile([C, CHUNK], f32)
                nc.scalar.activation(
                    t1[:],
                    psum_tile[0:C],
                    mybir.ActivationFunctionType.Identity,
                    bias=bias_tile[:],
                )
                nc.vector.scalar_tensor_tensor(
                    t2[:], psum_tile[32 : 32 + C], 1.0, t1[:], mult, add
                )
                nc.vector.scalar_tensor_tensor(
                    t1[:], psum_tile[64 : 64 + C], 1.0, t2[:], mult, add
                )
                nc.vector.scalar_tensor_tensor(
                    out_tile[:, ts(j % GROUP, CHUNK)],
                    psum_tile[96 : 96 + C],
                    1.0,
                    t1[:],
                    mult,
                    add,
                )
                if j % GROUP == GROUP - 1:
                    # stores ride the Scalar HWDGE ring, never stalling the
                    # Sync ring that feeds the streaming loads
                    nc.scalar.dma_start(
                        out=outT[:, ts(j // GROUP, GROUP * CHUNK)], in_=out_tile[:]
                    )

    nc.compile()
    return nc


def _fold_E(node_0, node_1, node_2) -> np.ndarray:
    # E[(i,j,k,l), c3] = sum_{c1,c2} node_0[l,k,c1] node_1[c1,j,c2] node_2[c2,i,c3]
    E = np.einsum(
        "lkc,cjd,die->ijkle",
        node_0.astype(np.float64),
        node_1.astype(np.float64),
        node_2.astype(np.float64),
    )
    return E.reshape(K, C).astype(np.float32)


def kernel(x, node_0, node_1, node_2, bias, _trace=False, _trace_cores=None):
    import ml_dtypes

    x = np.asarray(x, dtype=np.float32)
    E = _fold_E(np.asarray(node_0), np.asarray(node_1), np.asarray(node_2))
    bias_np = np.asarray(bias, dtype=np.float32).reshape(C, 1)

    # blocked E: eb[p, kt, c] = E[kt*128 + p, c]
    eb = np.ascontiguousarray(E.reshape(NK, 128, C).transpose(1, 0, 2)).astype(
        ml_dtypes.bfloat16
    )

    if "v2" not in _program_cache:
        _program_cache["v2"] = _build_program()
    nc = _program_cache["v2"]

    # quantize once in the natural layout, then do the (cheaper) 1-byte
    # blocked transpose per core
    x8 = x.astype(ml_dtypes.float8_e3m4)

    in_maps = []
    for m in range(N_CORES):
        x_m = x8[m * B_CORE : (m + 1) * B_CORE, :]
        # xb8[j, p, kt, b] = x_m[j*CHUNK + b, kt*128 + p]
        xb_m = np.ascontiguousarray(
            x_m.reshape(NCHUNK, CHUNK, NK, 128).transpose(0, 3, 2, 1)
        )
        in_maps.append({"xb8": xb_m, "eb": eb, "bias": bias_np})

    res = run_bass_kernel_spmd(
        nc,
        in_maps,
        core_ids=list(range(N_CORES)),
        trace=_trace,
        trace_cores=_trace_cores,
    )
    results = res.results

    out = np.empty((BATCH, C), dtype=np.float32)
    for m in range(N_CORES):
        out[m * B_CORE : (m + 1) * B_CORE, :] = results[m]["outT"].T

    if _trace:
        return out, res
    return out
